# revision 28
# baseline (speedup 1.0000x reference)
"""Trainium2 Bass kernel for nn_Attention_47777216200735.

Module: q = (Xq @ Wq.T + bq) * D^-0.5 ; k = Xk @ Wk.T + bk
        out = softmax(q @ k.T, axis=keys) @ k    (per batch/head; V == K)

Shapes: B=4, S=2048, DQ=DK=1024, H=16, D=64, fp32.

Sharding (8 NeuronCores): core c = (b, g) with b = c//2 (batch, data
parallel) and g = c%2 (head-group, tensor parallel: heads g*8..g*8+7 and
the matching 512 rows of Wq/Wk).  Attention is fully independent per
(b, h) so no collectives are needed; the host scatters inputs and
gathers/normalizes/transposes outputs.

Per-core graph (layouts chosen so no on-chip input transposes exist; the
host ships Xq.T, Xk.T, Wq_shard.T, Wk_shard.T):
  1. qT[c,s], kT[c,s] = W.T-tile.T @ X.T: bf16 matmuls, PSUM f32,
     evicted to SBUF as bf16 (eviction on the Scalar engine to keep the
     Vector engine free for exp work).
  2. k_ext[h,j] = PE-transpose of kT blocks -> [s,d] natural layout
     (bf16) plus a ones column (col 64) for the softmax denominator.
  3. heads processed in PAIRS (h0,h1) = one 128-partition channel tile;
     per 512-query chunk, per 128-key tile j:
       scoresT: TWO row-tiled matmuls (K=64, tile_position (0,0)/(64,0))
         run CONCURRENTLY in the PE array -> one [128,1024] PSUM unit.
       expT: ONE instruction over the whole unit.  Units are split
         between ScalarE (table exp) and VectorE (Schraudolph bit-trick
         exp: bf16_bits ~= int16(x*128/ln2 + B)) so the two engines
         share the exp load ~60/40.
       out'[d+1, i] += k_ext[h,j].T @ expT-half per head (kext
         stationary; ones column accumulates the denominator).
  4. Projection/transpose work for the NEXT pair is interleaved as PE
     filler at single-matmul granularity (a coarse 8-matmul block
     between the scores and out' of an iteration would stall the
     exp->out' dependency chain).
  5. DMA out' in [d+1, s] layout; host divides by row 64 and transposes.
"""

import ml_dtypes
import numpy as np
from contextlib import ExitStack

import concourse.bass as bass
import concourse.bacc as bacc
import concourse.tile as tile
import concourse.mybir as mybir
from concourse.bass_utils import run_bass_kernel_spmd

F32 = mybir.dt.float32
BF16 = mybir.dt.bfloat16
EXP = mybir.ActivationFunctionType.Exp

B, S, DQ, H, D = 4, 2048, 1024, 16, 64
P = 128
HC = H // 2          # heads per core = 8
C = HC * D           # projection channels per core = 512
KT = DQ // P         # 8 contraction tiles
CT = C // P          # 4 channel tiles (= head pairs)
NJ = S // P          # 16 key tiles
IC = 512             # query chunk (psum blocking)
NIC = S // IC        # 4
SCALE = float(D) ** -0.5
# Schraudolph bf16 exp on the vector engine: bf16_bits(exp(x)) ~=
# int16(x * 128/ln2 + B) (f32->int16 convert, bit-viewed as bf16).
# ~+-3% elementwise, washes out to ~2e-3 after softmax averaging.
# Each [128,1024] scores unit is exponentiated as TWO concurrent halves
# (h0 on ScalarE table-exp, h1 on VectorE Schraudolph) so the exp
# latency on the scores->exp->out' critical chain is ~690ns, not 1114.
SCHRAUD_A = 128.0 / float(np.log(2.0)) * SCALE
SCHRAUD_B = 16250.9

_CACHE: dict = {}
_last_in_maps = None


def _build(has_bias: bool):
    nc = bacc.Bacc("TRN2", target_bir_lowering=False, debug=False)

    xqt = nc.dram_tensor("xqt", [DQ, S], BF16, kind="ExternalInput").ap()
    xkt = nc.dram_tensor("xkt", [DQ, S], BF16, kind="ExternalInput").ap()
    wqt = nc.dram_tensor("wqt", [DQ, C], BF16, kind="ExternalInput").ap()
    wkt = nc.dram_tensor("wkt", [DQ, C], BF16, kind="ExternalInput").ap()
    if has_bias:
        bqr = nc.dram_tensor("bqr", [1, C], BF16, kind="ExternalInput").ap()
        bkr = nc.dram_tensor("bkr", [1, C], BF16, kind="ExternalInput").ap()
        onesd = nc.dram_tensor("onesd", [1, S], BF16, kind="ExternalInput").ap()
    idn = nc.dram_tensor("idn", [P, P], F32, kind="ExternalInput").ap()
    out = nc.dram_tensor("out", [HC, D + 1, S], F32, kind="ExternalOutput").ap()

    with tile.TileContext(nc) as tc, ExitStack() as ctx:
        const_p = ctx.enter_context(tc.tile_pool(name="const", bufs=1))
        w_p = ctx.enter_context(tc.tile_pool(name="wp", bufs=2 * KT))
        xq_p = ctx.enter_context(tc.tile_pool(name="xqp", bufs=KT))
        xk_p = ctx.enter_context(tc.tile_pool(name="xkp", bufs=KT))
        qk_p = ctx.enter_context(tc.tile_pool(name="qkp", bufs=CT))
        kext_p = ctx.enter_context(tc.tile_pool(name="kextp", bufs=1))
        exp_p = ctx.enter_context(tc.tile_pool(name="expp", bufs=4))
        ob_p = ctx.enter_context(tc.tile_pool(name="obp", bufs=4))
        # PSUM (8 banks): scores pair-units 2 x [128,1024]f32 (2 banks
        # each); out' accumulators 2 x [65,512]f32 (1 bank each); filler
        # (projection blocks + kext transposes) 2 x 1 bank.
        psumS = ctx.enter_context(tc.tile_pool(name="psS", bufs=2, space="PSUM"))
        psumB = ctx.enter_context(tc.tile_pool(name="psB", bufs=2, space="PSUM"))
        psumF = ctx.enter_context(tc.tile_pool(name="psF", bufs=2, space="PSUM"))

        identf = const_p.tile([P, P], F32)
        nc.sync.dma_start(out=identf[:], in_=idn[:])
        identb = const_p.tile([P, P], BF16)
        nc.vector.tensor_copy(identb[:], identf[:])

        # HAM warmup: ~3.3us of dummy PE transposes that only depend on
        # the (tiny, first) identity DMA — they run during the input-DMA
        # window and flip the PE clock gate to 8/8 before real work.
        for w in range(12):
            wt = psumF.tile([P, 1024], BF16, tag="fill", name=f"warm{w}")
            nc.tensor.transpose(wt[:, 0:P], identb[:], identb[:])

        # k_ext: one big tile, slices (h, j) -> [128 keys, 64 d + ones].
        # Block stride 128: the DMA-crossbar transpose only writes
        # correctly at destination offsets of 0/32 mod 128 elements
        # (measured); 128 keeps every block on a safe boundary.
        KCOL = 2 * D
        kext = kext_p.tile([P, HC * NJ * KCOL], BF16)
        nc.gpsimd.memset(kext[:], 1.0)

        def kx(h, j):
            o = (h * NJ + j) * KCOL
            return kext[:, o:o + D + 1]

        if has_bias:
            ones_sb = const_p.tile([1, S], BF16)
            nc.sync.dma_start(out=ones_sb[:], in_=onesd[:])
            bq_sb = const_p.tile([1, C], BF16)
            bk_sb = const_p.tile([1, C], BF16)
            nc.sync.dma_start(out=bq_sb[:], in_=bqr[:])
            nc.sync.dma_start(out=bk_sb[:], in_=bkr[:])

        # ---- weights + inputs: k-side interleaved first on the sync
        # queue (its first projection matmul can start after ~0.6MB), the
        # q-side on the scalar queue so it doesn't delay the k-side.
        w_tiles = {}
        xk, xq = [], []
        for kt in range(KT):
            t = w_p.tile([P, C], BF16, tag="w", name=f"wk{kt}")
            nc.sync.dma_start(out=t[:], in_=wkt[kt * P:(kt + 1) * P, :])
            w_tiles["k", kt] = t
            t2 = xk_p.tile([P, S], BF16, tag="x", name=f"xk{kt}")
            nc.sync.dma_start(out=t2[:], in_=xkt[kt * P:(kt + 1) * P, :])
            xk.append(t2)
        for kt in range(KT):
            t = w_p.tile([P, C], BF16, tag="w", name=f"wq{kt}")
            nc.scalar.dma_start(out=t[:], in_=wqt[kt * P:(kt + 1) * P, :])
            w_tiles["q", kt] = t
            t2 = xq_p.tile([P, S], BF16, tag="x", name=f"xq{kt}")
            nc.scalar.dma_start(out=t2[:], in_=xqt[kt * P:(kt + 1) * P, :])
            xq.append(t2)

        qk_tiles = {}

        def get_qk(name, ct):
            if (name, ct) not in qk_tiles:
                qk_tiles[name, ct] = qk_p.tile(
                    [P, S], BF16, tag=f"qk_{name}", name=f"{name}T{ct}")
            return qk_tiles[name, ct]

        def proj_granules(name, ct, sb, xt, bias_sb, pool=None, tag="fill",
                          evict_engine=None):
            """One [128,512] projection block as per-matmul granules."""
            state = {}
            n_acc = KT + (1 if has_bias else 0)

            def mm(kt):
                if kt == 0:
                    state["ps"] = (pool or psumF).tile(
                        [P, 512], F32, tag=tag, name=f"ps{name}{ct}{sb}")
                nc.tensor.matmul(
                    state["ps"][:],
                    lhsT=w_tiles[name, kt][:, ct * P:(ct + 1) * P],
                    rhs=xt[kt][:, sb * 512:(sb + 1) * 512],
                    start=(kt == 0),
                    stop=(kt == n_acc - 1),
                )

            def bias_mm():
                nc.tensor.matmul(
                    state["ps"][:],
                    lhsT=bias_sb[:, ct * P:(ct + 1) * P],
                    rhs=ones_sb[:, sb * 512:(sb + 1) * 512],
                    start=False, stop=True,
                )

            def evict():
                dst = get_qk(name, ct)
                eng = evict_engine or nc.vector
                if eng is nc.scalar:
                    eng.copy(dst[:, sb * 512:(sb + 1) * 512], state["ps"][:])
                else:
                    eng.tensor_copy(dst[:, sb * 512:(sb + 1) * 512],
                                    state["ps"][:])

            mms = [lambda kt=kt: mm(kt) for kt in range(KT)]
            if has_bias:
                mms.append(bias_mm)
            return mms, evict

        def kext_granule(ct, j, pool=None, tag="fill"):
            # DMA-crossbar transpose kicked from the sync queue (the only
            # hw-DGE queue with slack): costs the PE and DVE nothing.
            kT = qk_tiles["k", ct]
            nc.sync.dma_start_transpose(
                out=kx(2 * ct, j)[:, 0:D], in_=kT[0:D, j * P:(j + 1) * P])
            nc.sync.dma_start_transpose(
                out=kx(2 * ct + 1, j)[:, 0:D], in_=kT[D:P, j * P:(j + 1) * P])

        def ct_granules(ct, pool=None, tag="fill", evict_engine=None):
            """Filler granules preparing channel-tile ct, ordered so
            every granule sits well after the granules it depends on
            (evictions one block behind their matmuls, transposes long
            after the eviction they read) — an in-order engine queue
            head-of-line-blocks on a granule whose input isn't ready."""
            bk = bk_sb if has_bias else None
            bq = bq_sb if has_bias else None
            kmm, kev = [], []
            qmm, qev = [], []
            for sb in range(S // 512):
                a, b = proj_granules("k", ct, sb, xk, bk, pool, tag,
                                     evict_engine)
                kmm.append(a)
                kev.append(b)
                a, b = proj_granules("q", ct, sb, xq, bq, pool, tag,
                                     evict_engine)
                qmm.append(a)
                qev.append(b)
            kxg = [lambda j=j: kext_granule(ct, j, pool, tag)
                   for j in range(NJ)]
            seq = (kmm[0] + kmm[1] + [kev[0]] + kmm[2] + [kev[1]]
                   + kmm[3] + [kev[2]] + qmm[0] + [kev[3]]
                   + kxg[0:4] + qmm[1] + [qev[0]]
                   + kxg[4:8] + qmm[2] + [qev[1]]
                   + kxg[8:12] + qmm[3] + [qev[2]]
                   + kxg[12:16] + [qev[3]])
            yield from seq

        N_GRAN = 2 * (S // 512) * (KT + (1 if has_bias else 0) + 1) + NJ

        def attention_pair(ct, filler):
            """Attention for head pair (2ct, 2ct+1); scores row-tiled
            pairs run concurrently; exp alternates ScalarE/VectorE;
            filler granules are paced into the exp-wait window between
            the scores and out' matmuls of each iteration."""
            qT = qk_tiles["q", ct]
            kT = qk_tiles["k", ct]
            h0, h1 = 2 * ct, 2 * ct + 1
            n_iter = NIC * (NJ + 1)
            it = 0
            emitted = 0
            for ic in range(NIC):
                cs = slice(ic * IC, (ic + 1) * IC)
                acc0 = psumB.tile([D + 1, IC], F32, tag="acc", name=f"a0_{ct}_{ic}")
                acc1 = psumB.tile([D + 1, IC], F32, tag="acc", name=f"a1_{ct}_{ic}")
                sp_prev = None
                for j in range(NJ + 1):
                    if j < NJ:
                        sp = psumS.tile([P, 2 * IC], F32, tag="sc",
                                        name=f"sp{ct}_{ic}_{j}")
                        nc.tensor.matmul(
                            sp[:, 0:IC],
                            lhsT=kT[0:D, j * P:(j + 1) * P],
                            rhs=qT[0:D, cs], start=True, stop=True)
                        nc.tensor.matmul(
                            sp[:, IC:2 * IC],
                            lhsT=kT[D:P, j * P:(j + 1) * P],
                            rhs=qT[D:P, cs], start=True, stop=True)
                    if j > 0:
                        jj = j - 1
                        et = exp_p.tile([P, 2 * IC], BF16, tag="exp",
                                        name=f"et{ct}_{ic}_{jj}")
                        nc.scalar.activation(et[:, 0:IC], sp_prev[:, 0:IC],
                                             EXP, scale=SCALE)
                        nc.vector.tensor_scalar(
                            out=et[:, IC:2 * IC].bitcast(mybir.dt.int16),
                            in0=sp_prev[:, IC:2 * IC],
                            scalar1=SCHRAUD_A, scalar2=SCHRAUD_B,
                            op0=mybir.AluOpType.mult,
                            op1=mybir.AluOpType.add)
                        # filler granules inside the exp-wait window
                        it += 1
                        want = min((it * N_GRAN) // n_iter, emitted + 2)
                        while emitted < want:
                            u = next(filler, None)
                            if u is None:
                                break
                            u()
                            emitted += 1
                        nc.tensor.matmul(
                            acc0[:], lhsT=kx(h0, jj)[:], rhs=et[:, 0:IC],
                            start=(jj == 0), stop=(jj == NJ - 1))
                        nc.tensor.matmul(
                            acc1[:], lhsT=kx(h1, jj)[:], rhs=et[:, IC:2 * IC],
                            start=(jj == 0), stop=(jj == NJ - 1))
                    else:
                        it += 1
                    sp_prev = sp
                ob0 = ob_p.tile([D + 1, IC], F32, tag="ob", name=f"ob0_{ct}_{ic}")
                nc.scalar.copy(ob0[:], acc0[:])
                nc.sync.dma_start(out=out[h0, :, cs], in_=ob0[:])
                ob1 = ob_p.tile([D + 1, IC], F32, tag="ob", name=f"ob1_{ct}_{ic}")
                nc.scalar.copy(ob1[:], acc1[:])
                nc.sync.dma_start(out=out[h1, :, cs], in_=ob1[:])

        # ---- emission: ct=0 prepared densely (lead-in, cycling all
        # psum pools so blocks pipeline), then each pair's attention
        # paces the NEXT channel-tile's granules as PE filler.
        pools = [(psumS, "sc"), (psumF, "fill"), (psumB, "acc")]
        bkb = bk_sb if has_bias else None
        bqb = bq_sb if has_bias else None
        pre = []
        for sb in range(S // 512):
            pl = pools[sb % 3]
            mms, ev = proj_granules("k", 0, sb, xk, bkb, pl[0], pl[1],
                                    nc.vector)
            pre.extend(mms)
            pre.append(ev)
        for j in range(NJ):
            pl = pools[j % 3]
            pre.append(lambda j=j, pl=pl: kext_granule(0, j, pl[0], pl[1]))
        for sb in range(S // 512):
            pl = pools[(sb + 1) % 3]
            mms, ev = proj_granules("q", 0, sb, xq, bqb, pl[0], pl[1],
                                    nc.vector)
            pre.extend(mms)
            pre.append(ev)
        for u in pre:
            u()
        for ct in range(CT):
            gran = ct_granules(ct + 1) if ct + 1 < CT else iter(())
            attention_pair(ct, gran)
            for u in gran:   # drain any remainder
                u()

    nc.compile()
    return nc


def _transposed(x):
    return np.ascontiguousarray(np.asarray(x, dtype=np.float32).T
                                ).astype(ml_dtypes.bfloat16)


def kernel(query_input, key_input, Wq, bq, Wk, bk):
    query_input = np.asarray(query_input, dtype=np.float32)
    key_input = np.asarray(key_input, dtype=np.float32)
    Wq = np.asarray(Wq, dtype=np.float32)
    Wk = np.asarray(Wk, dtype=np.float32)
    bq = np.asarray(bq, dtype=np.float32)
    bk = np.asarray(bk, dtype=np.float32)

    has_bias = bool(np.any(bq) or np.any(bk))
    if ("nc", has_bias) not in _CACHE:
        _CACHE["nc", has_bias] = _build(has_bias)
    nc = _CACHE["nc", has_bias]

    in_maps = []
    for c in range(8):
        b, g = divmod(c, 2)
        rows = slice(g * C, (g + 1) * C)
        m = {
            "idn": np.eye(P, dtype=np.float32),
            "xqt": _transposed(query_input[b]),
            "xkt": _transposed(key_input[b]),
            "wqt": _transposed(Wq[rows]),
            "wkt": _transposed(Wk[rows]),
        }
        if has_bias:
            m["bqr"] = np.ascontiguousarray(bq[rows])[None, :].astype(ml_dtypes.bfloat16)
            m["bkr"] = np.ascontiguousarray(bk[rows])[None, :].astype(ml_dtypes.bfloat16)
            m["onesd"] = np.ones((1, S), dtype=ml_dtypes.bfloat16)
        in_maps.append(m)

    global _last_in_maps
    _last_in_maps = in_maps
    res = run_bass_kernel_spmd(nc, in_maps, core_ids=list(range(8)))

    full = np.empty((B, S, H * D), dtype=np.float32)
    for c in range(8):
        b, g = divmod(c, 2)
        o = res.results[c]["out"]                    # [HC, D+1, S]
        o = o[:, :D, :] / o[:, D:D + 1, :]           # softmax normalization
        full[b, :, g * C:(g + 1) * C] = o.transpose(2, 0, 1).reshape(S, C)
    return full


# revision 33
# speedup vs baseline: 1.1852x; 1.1852x over previous
"""Trainium2 Bass kernel for nn_Attention_47777216200735.

Module: q = (Xq @ Wq.T + bq) * D^-0.5 ; k = Xk @ Wk.T + bk
        out = softmax(q @ k.T, axis=keys) @ k    (per batch/head; V == K)

Shapes: B=4, S=2048, DQ=DK=1024, H=16, D=64, fp32.

Sharding (8 NeuronCores): core c = (b, g) with b = c//2 (batch, data
parallel) and g = c%2 (head-group, tensor parallel: heads g*8..g*8+7 and
the matching 512 rows of Wq/Wk).  Attention is fully independent per
(b, h) so no collectives are needed; the host scatters inputs and
gathers/normalizes/transposes outputs.

Per-core graph (layouts chosen so no on-chip input transposes exist; the
host ships Xq.T, Xk.T, Wq_shard.T, Wk_shard.T):
  1. qT[c,s], kT[c,s] = W.T-tile.T @ X.T: bf16 matmuls, PSUM f32,
     evicted to SBUF as bf16 (eviction on the Scalar engine to keep the
     Vector engine free for exp work).
  2. k_ext[h,j] = PE-transpose of kT blocks -> [s,d] natural layout
     (bf16) plus a ones column (col 64) for the softmax denominator.
  3. heads processed in PAIRS (h0,h1) = one 128-partition channel tile;
     per 512-query chunk, per 128-key tile j:
       scoresT: TWO row-tiled matmuls (K=64, tile_position (0,0)/(64,0))
         run CONCURRENTLY in the PE array -> one [128,1024] PSUM unit.
       expT: ONE instruction over the whole unit.  Units are split
         between ScalarE (table exp) and VectorE (Schraudolph bit-trick
         exp: bf16_bits ~= int16(x*128/ln2 + B)) so the two engines
         share the exp load ~60/40.
       out'[d+1, i] += k_ext[h,j].T @ expT-half per head (kext
         stationary; ones column accumulates the denominator).
  4. Projection/transpose work for the NEXT pair is interleaved as PE
     filler at single-matmul granularity (a coarse 8-matmul block
     between the scores and out' of an iteration would stall the
     exp->out' dependency chain).
  5. DMA out' in [d+1, s] layout; host divides by row 64 and transposes.
"""

import ml_dtypes
import numpy as np
from contextlib import ExitStack

import concourse.bass as bass
import concourse.bacc as bacc
import concourse.tile as tile
import concourse.mybir as mybir
from concourse.bass_utils import run_bass_kernel_spmd

F32 = mybir.dt.float32
BF16 = mybir.dt.bfloat16
EXP = mybir.ActivationFunctionType.Exp

B, S, DQ, H, D = 4, 2048, 1024, 16, 64
P = 128
HC = H // 2          # heads per core = 8
C = HC * D           # projection channels per core = 512
KT = DQ // P         # 8 contraction tiles
CT = C // P          # 4 channel tiles (= head pairs)
NJ = S // P          # 16 key tiles
IC = 512             # query chunk (psum blocking)
NIC = S // IC        # 4
SCALE = float(D) ** -0.5
# Schraudolph bf16 exp on the vector engine: bf16_bits(exp(x)) ~=
# int16(x * 128/ln2 + B) (f32->int16 convert, bit-viewed as bf16).
# ~+-3% elementwise, washes out to ~2e-3 after softmax averaging.
SCHRAUD_A = 128.0 / float(np.log(2.0)) * SCALE
SCHRAUD_B = 16250.9
DVE_J = frozenset((2, 4, 7, 9, 12, 14))   # key tiles whose exp runs on DVE

_CACHE: dict = {}
_last_in_maps = None


def _build(has_bias: bool):
    nc = bacc.Bacc("TRN2", target_bir_lowering=False, debug=False)

    xqt = nc.dram_tensor("xqt", [DQ, S], BF16, kind="ExternalInput").ap()
    xkt = nc.dram_tensor("xkt", [DQ, S], BF16, kind="ExternalInput").ap()
    wqt = nc.dram_tensor("wqt", [DQ, C], BF16, kind="ExternalInput").ap()
    wkt = nc.dram_tensor("wkt", [DQ, C], BF16, kind="ExternalInput").ap()
    if has_bias:
        bqr = nc.dram_tensor("bqr", [1, C], BF16, kind="ExternalInput").ap()
        bkr = nc.dram_tensor("bkr", [1, C], BF16, kind="ExternalInput").ap()
        onesd = nc.dram_tensor("onesd", [1, S], BF16, kind="ExternalInput").ap()
    idn = nc.dram_tensor("idn", [P, P], F32, kind="ExternalInput").ap()
    out = nc.dram_tensor("out", [HC, D + 1, S], F32, kind="ExternalOutput").ap()

    with tile.TileContext(nc) as tc, ExitStack() as ctx:
        const_p = ctx.enter_context(tc.tile_pool(name="const", bufs=1))
        w_p = ctx.enter_context(tc.tile_pool(name="wp", bufs=2 * KT))
        xq_p = ctx.enter_context(tc.tile_pool(name="xqp", bufs=KT))
        xk_p = ctx.enter_context(tc.tile_pool(name="xkp", bufs=KT))
        qk_p = ctx.enter_context(tc.tile_pool(name="qkp", bufs=CT))
        kext_p = ctx.enter_context(tc.tile_pool(name="kextp", bufs=1))
        exp_p = ctx.enter_context(tc.tile_pool(name="expp", bufs=4))
        ob_p = ctx.enter_context(tc.tile_pool(name="obp", bufs=4))
        # PSUM (8 banks): scores pair-units 2 x [128,1024]f32 (2 banks
        # each); out' accumulators 2 x [65,512]f32 (1 bank each); filler
        # (projection blocks + kext transposes) 2 x 1 bank.
        psumS = ctx.enter_context(tc.tile_pool(name="psS", bufs=2, space="PSUM"))
        psumB = ctx.enter_context(tc.tile_pool(name="psB", bufs=2, space="PSUM"))
        psumF = ctx.enter_context(tc.tile_pool(name="psF", bufs=2, space="PSUM"))

        identf = const_p.tile([P, P], F32)
        nc.sync.dma_start(out=identf[:], in_=idn[:])
        identb = const_p.tile([P, P], BF16)
        nc.vector.tensor_copy(identb[:], identf[:])

        # HAM warmup: ~3.3us of dummy PE transposes that only depend on
        # the (tiny, first) identity DMA — they run during the input-DMA
        # window and flip the PE clock gate to 8/8 before real work.
        for w in range(12):
            wt = psumF.tile([P, 1024], BF16, tag="fill", name=f"warm{w}")
            nc.tensor.transpose(wt[:, 0:P], identb[:], identb[:])

        # k_ext: one big tile, slices (h, j) -> [128 keys, 64 d + ones].
        # Block stride 128: the DMA-crossbar transpose only writes
        # correctly at destination offsets of 0/32 mod 128 elements
        # (measured); 128 keeps every block on a safe boundary.
        KCOL = 2 * D
        kext = kext_p.tile([P, HC * NJ * KCOL], BF16)
        nc.gpsimd.memset(kext[:], 1.0)

        def kx(h, j):
            o = (h * NJ + j) * KCOL
            return kext[:, o:o + D + 1]

        if has_bias:
            ones_sb = const_p.tile([1, S], BF16)
            nc.sync.dma_start(out=ones_sb[:], in_=onesd[:])
            bq_sb = const_p.tile([1, C], BF16)
            bk_sb = const_p.tile([1, C], BF16)
            nc.sync.dma_start(out=bq_sb[:], in_=bqr[:])
            nc.sync.dma_start(out=bk_sb[:], in_=bkr[:])

        # ---- weights + inputs: k-side interleaved first on the sync
        # queue (its first projection matmul can start after ~0.6MB), the
        # q-side on the scalar queue so it doesn't delay the k-side.
        w_tiles = {}
        xk, xq = [], []
        for kt in range(KT):
            t = w_p.tile([P, C], BF16, tag="w", name=f"wk{kt}")
            nc.sync.dma_start(out=t[:], in_=wkt[kt * P:(kt + 1) * P, :])
            w_tiles["k", kt] = t
            t2 = xk_p.tile([P, S], BF16, tag="x", name=f"xk{kt}")
            nc.sync.dma_start(out=t2[:], in_=xkt[kt * P:(kt + 1) * P, :])
            xk.append(t2)
        for kt in range(KT):
            t = w_p.tile([P, C], BF16, tag="w", name=f"wq{kt}")
            nc.scalar.dma_start(out=t[:], in_=wqt[kt * P:(kt + 1) * P, :])
            w_tiles["q", kt] = t
            t2 = xq_p.tile([P, S], BF16, tag="x", name=f"xq{kt}")
            nc.scalar.dma_start(out=t2[:], in_=xqt[kt * P:(kt + 1) * P, :])
            xq.append(t2)

        qk_tiles = {}

        def get_qk(name, ct):
            if (name, ct) not in qk_tiles:
                qk_tiles[name, ct] = qk_p.tile(
                    [P, S], BF16, tag=f"qk_{name}", name=f"{name}T{ct}")
            return qk_tiles[name, ct]

        def proj_granules(name, ct, sb, xt, bias_sb, pool=None, tag="fill",
                          evict_engine=None):
            """One [128,512] projection block as per-matmul granules."""
            state = {}
            n_acc = KT + (1 if has_bias else 0)

            def mm(kt):
                if kt == 0:
                    state["ps"] = (pool or psumF).tile(
                        [P, 512], F32, tag=tag, name=f"ps{name}{ct}{sb}")
                nc.tensor.matmul(
                    state["ps"][:],
                    lhsT=w_tiles[name, kt][:, ct * P:(ct + 1) * P],
                    rhs=xt[kt][:, sb * 512:(sb + 1) * 512],
                    start=(kt == 0),
                    stop=(kt == n_acc - 1),
                )

            def bias_mm():
                nc.tensor.matmul(
                    state["ps"][:],
                    lhsT=bias_sb[:, ct * P:(ct + 1) * P],
                    rhs=ones_sb[:, sb * 512:(sb + 1) * 512],
                    start=False, stop=True,
                )

            def evict():
                dst = get_qk(name, ct)
                eng = evict_engine or nc.vector
                if eng is nc.scalar:
                    eng.copy(dst[:, sb * 512:(sb + 1) * 512], state["ps"][:])
                else:
                    eng.tensor_copy(dst[:, sb * 512:(sb + 1) * 512],
                                    state["ps"][:])

            mms = [lambda kt=kt: mm(kt) for kt in range(KT)]
            if has_bias:
                mms.append(bias_mm)
            return mms, evict

        def kext_granule(ct, j, pool=None, tag="fill"):
            tp = (pool or psumF).tile([P, P], BF16, tag=tag,
                                      name=f"tp{ct}_{j}")
            nc.tensor.transpose(
                tp[:], qk_tiles["k", ct][:, j * P:(j + 1) * P], identb[:]
            )
            nc.vector.tensor_copy(kx(2 * ct, j)[:, 0:D], tp[:, 0:D])
            nc.vector.tensor_copy(kx(2 * ct + 1, j)[:, 0:D], tp[:, D:P])

        def ct_granules(ct, pool=None, tag="fill", evict_engine=None):
            """Filler granules preparing channel-tile ct, ordered so
            every granule sits well after the granules it depends on
            (evictions one block behind their matmuls, transposes long
            after the eviction they read) — an in-order engine queue
            head-of-line-blocks on a granule whose input isn't ready."""
            bk = bk_sb if has_bias else None
            bq = bq_sb if has_bias else None
            kmm, kev = [], []
            qmm, qev = [], []
            for sb in range(S // 512):
                a, b = proj_granules("k", ct, sb, xk, bk, pool, tag,
                                     evict_engine)
                kmm.append(a)
                kev.append(b)
                a, b = proj_granules("q", ct, sb, xq, bq, pool, tag,
                                     evict_engine)
                qmm.append(a)
                qev.append(b)
            kxg = [lambda j=j: kext_granule(ct, j, pool, tag)
                   for j in range(NJ)]
            seq = (kmm[0] + kmm[1] + [kev[0]] + kmm[2] + [kev[1]]
                   + kmm[3] + [kev[2]] + qmm[0] + [kev[3]]
                   + kxg[0:4] + qmm[1] + [qev[0]]
                   + kxg[4:8] + qmm[2] + [qev[1]]
                   + kxg[8:12] + qmm[3] + [qev[2]]
                   + kxg[12:16] + [qev[3]])
            yield from seq

        N_GRAN = 2 * (S // 512) * (KT + (1 if has_bias else 0) + 1) + NJ

        def attention_pair(ct, filler):
            """Attention for head pair (2ct, 2ct+1); scores row-tiled
            pairs run concurrently; exp alternates ScalarE/VectorE;
            filler granules are paced into the exp-wait window between
            the scores and out' matmuls of each iteration."""
            qT = qk_tiles["q", ct]
            kT = qk_tiles["k", ct]
            h0, h1 = 2 * ct, 2 * ct + 1
            n_iter = NIC * (NJ + 2)
            it = 0
            emitted = 0
            for ic in range(NIC):
                cs = slice(ic * IC, (ic + 1) * IC)
                acc0 = psumB.tile([D + 1, IC], F32, tag="acc", name=f"a0_{ct}_{ic}")
                acc1 = psumB.tile([D + 1, IC], F32, tag="acc", name=f"a1_{ct}_{ic}")
                sps = {}
                ets = {}
                # software pipeline depth 2: iteration j issues scores(j),
                # exp(j-1) and out'(j-2), so out' consumes an exp that
                # completed a full iteration ago (no latency coupling).
                for j in range(NJ + 2):
                    if j < NJ:
                        sp = psumS.tile([P, 2 * IC], F32, tag="sc",
                                        name=f"sp{ct}_{ic}_{j}")
                        sps[j] = sp
                        nc.tensor.matmul(
                            sp[:, 0:IC],
                            lhsT=kT[0:D, j * P:(j + 1) * P],
                            rhs=qT[0:D, cs], start=True, stop=True)
                        nc.tensor.matmul(
                            sp[:, IC:2 * IC],
                            lhsT=kT[D:P, j * P:(j + 1) * P],
                            rhs=qT[D:P, cs], start=True, stop=True)
                    if 1 <= j <= NJ:
                        je = j - 1
                        et = exp_p.tile([P, 2 * IC], BF16, tag="exp",
                                        name=f"et{ct}_{ic}_{je}")
                        ets[je] = et
                        spe = sps.pop(je)
                        if je in DVE_J:
                            nc.vector.tensor_scalar(
                                out=et[:].bitcast(mybir.dt.int16),
                                in0=spe[:],
                                scalar1=SCHRAUD_A, scalar2=SCHRAUD_B,
                                op0=mybir.AluOpType.mult,
                                op1=mybir.AluOpType.add)
                        else:
                            nc.scalar.activation(et[:], spe[:], EXP,
                                                 scale=SCALE)
                    # filler granules inside the exp pipeline slack
                    it += 1
                    want = min((it * N_GRAN) // n_iter, emitted + 2)
                    while emitted < want:
                        u = next(filler, None)
                        if u is None:
                            break
                        u()
                        emitted += 1
                    if j >= 2:
                        jj = j - 2
                        et = ets.pop(jj)
                        nc.tensor.matmul(
                            acc0[:], lhsT=kx(h0, jj)[:], rhs=et[:, 0:IC],
                            start=(jj == 0), stop=(jj == NJ - 1))
                        nc.tensor.matmul(
                            acc1[:], lhsT=kx(h1, jj)[:], rhs=et[:, IC:2 * IC],
                            start=(jj == 0), stop=(jj == NJ - 1))
                ob0 = ob_p.tile([D + 1, IC], F32, tag="ob", name=f"ob0_{ct}_{ic}")
                nc.vector.tensor_copy(ob0[:], acc0[:])
                nc.sync.dma_start(out=out[h0, :, cs], in_=ob0[:])
                ob1 = ob_p.tile([D + 1, IC], F32, tag="ob", name=f"ob1_{ct}_{ic}")
                nc.vector.tensor_copy(ob1[:], acc1[:])
                nc.sync.dma_start(out=out[h1, :, cs], in_=ob1[:])

        # ---- emission: ct=0 prepared densely (lead-in, cycling all
        # psum pools so blocks pipeline), then each pair's attention
        # paces the NEXT channel-tile's granules as PE filler.
        pools = [(psumS, "sc"), (psumF, "fill"), (psumB, "acc")]
        bkb = bk_sb if has_bias else None
        bqb = bq_sb if has_bias else None
        pre = []
        for sb in range(S // 512):
            pl = pools[sb % 3]
            mms, ev = proj_granules("k", 0, sb, xk, bkb, pl[0], pl[1],
                                    nc.vector)
            pre.extend(mms)
            pre.append(ev)
        for j in range(NJ):
            pl = pools[j % 3]
            pre.append(lambda j=j, pl=pl: kext_granule(0, j, pl[0], pl[1]))
        for sb in range(S // 512):
            pl = pools[(sb + 1) % 3]
            mms, ev = proj_granules("q", 0, sb, xq, bqb, pl[0], pl[1],
                                    nc.vector)
            pre.extend(mms)
            pre.append(ev)
        for u in pre:
            u()
        for ct in range(CT):
            gran = ct_granules(ct + 1) if ct + 1 < CT else iter(())
            attention_pair(ct, gran)
            for u in gran:   # drain any remainder
                u()

    nc.compile()
    return nc


def _transposed(x):
    return np.ascontiguousarray(np.asarray(x, dtype=np.float32).T
                                ).astype(ml_dtypes.bfloat16)


def kernel(query_input, key_input, Wq, bq, Wk, bk):
    query_input = np.asarray(query_input, dtype=np.float32)
    key_input = np.asarray(key_input, dtype=np.float32)
    Wq = np.asarray(Wq, dtype=np.float32)
    Wk = np.asarray(Wk, dtype=np.float32)
    bq = np.asarray(bq, dtype=np.float32)
    bk = np.asarray(bk, dtype=np.float32)

    has_bias = bool(np.any(bq) or np.any(bk))
    if ("nc", has_bias) not in _CACHE:
        _CACHE["nc", has_bias] = _build(has_bias)
    nc = _CACHE["nc", has_bias]

    in_maps = []
    for c in range(8):
        b, g = divmod(c, 2)
        rows = slice(g * C, (g + 1) * C)
        m = {
            "idn": np.eye(P, dtype=np.float32),
            "xqt": _transposed(query_input[b]),
            "xkt": _transposed(key_input[b]),
            "wqt": _transposed(Wq[rows]),
            "wkt": _transposed(Wk[rows]),
        }
        if has_bias:
            m["bqr"] = np.ascontiguousarray(bq[rows])[None, :].astype(ml_dtypes.bfloat16)
            m["bkr"] = np.ascontiguousarray(bk[rows])[None, :].astype(ml_dtypes.bfloat16)
            m["onesd"] = np.ones((1, S), dtype=ml_dtypes.bfloat16)
        in_maps.append(m)

    global _last_in_maps
    _last_in_maps = in_maps
    res = run_bass_kernel_spmd(nc, in_maps, core_ids=list(range(8)))

    full = np.empty((B, S, H * D), dtype=np.float32)
    for c in range(8):
        b, g = divmod(c, 2)
        o = res.results[c]["out"]                    # [HC, D+1, S]
        o = o[:, :D, :] / o[:, D:D + 1, :]           # softmax normalization
        full[b, :, g * C:(g + 1) * C] = o.transpose(2, 0, 1).reshape(S, C)
    return full
